# revision 4
# baseline (speedup 1.0000x reference)
"""Trainium2 Bass kernel for nn_CompressiveMemory_57750130262084.

The reference computes (B=8, S=4096, DK=DV=1024):
    sigma  = elu(query) + 1                                  [B,S,DK]
    memory = einsum('bkd,bsv->bkv', swap(sigma), value)      [B,DK,DV]
    z_norm = sum_s sigma                                     [B,DK]
    out    = einsum('bsd,bkv->bsv', sigma, memory)
           / einsum('bsd,bk->bs',  sigma, z_norm)[..., None]

Every einsum uses disjoint summed subscripts, so each factorises into
outer products of independent reductions:
    memory[b,k,v]    = z_norm[b,k] * VS[b,v]      with VS[b,v] = sum_s value[b,s,v]
    retrieved[b,s,v] = rs[b,s] * Z[b] * VS[b,v]   with rs = rowsum(sigma), Z = sum_k z_norm
    denom[b,s]       = rs[b,s] * Z[b]
    out[b,s,v]       = VS[b,v]                    (exactly; query cancels)

So the kernel is a column-sum of `value` over S, broadcast over S.
Sharding: data-parallel over batch, one NeuronCore per batch element.
Per-core work: read 16 MB, reduce 4096 rows -> 1 row, write 16 MB.
"""

import numpy as np

B, S, D = 8, 4096, 1024
P = 128                 # SBUF partitions
N_CHUNK = S // P        # 32 row-chunks of 128 rows
IN_REP = 8              # row-chunks per input DMA  -> [128, 8*1024] = 4 MB tiles
OUT_REP = 8             # row-chunks per output DMA -> [128, 8*1024] = 4 MB writes
N_IN = N_CHUNK // IN_REP
N_OUT = N_CHUNK // OUT_REP

_CACHE: dict = {}


def _build_program():
    import concourse.mybir as mybir
    import concourse.tile as tile
    from concourse import bacc

    f32 = mybir.dt.float32
    nc = bacc.Bacc("TRN2", target_bir_lowering=False, debug=False, num_devices=B)
    v = nc.declare_dram_parameter("value", [S, D], f32, isOutput=False)
    o = nc.declare_dram_parameter("out", [S, D], f32, isOutput=True)

    # DRAM views: [i][p][n][m] with row = (i*REP + n)*128 + p
    v_re = v[:].rearrange("(i n p) m -> i p n m", i=N_IN, n=IN_REP, p=P)
    o_re = o[:].rearrange("(i n p) m -> i p n m", i=N_OUT, n=OUT_REP, p=P)

    with tile.TileContext(nc) as tc:
        with (
            tc.tile_pool(name="in", bufs=N_IN) as in_pool,
            tc.tile_pool(name="acc", bufs=1) as acc_pool,
            tc.tile_pool(name="ones", bufs=1) as ones_pool,
            tc.tile_pool(name="bcast", bufs=1) as bcast_pool,
            tc.tile_pool(name="psum", bufs=1, space="PSUM") as psum_pool,
        ):
            ones = ones_pool.tile([P, P], f32)
            nc.vector.memset(ones[:], 1.0)

            acc = acc_pool.tile([P, D], f32)
            first = True
            for i in range(N_IN):
                t = in_pool.tile([P, IN_REP * D], f32)
                nc.sync.dma_start(
                    t[:].rearrange("p (n m) -> p n m", n=IN_REP), v_re[i]
                )
                for n in range(IN_REP):
                    sl = t[:, n * D : (n + 1) * D]
                    if first:
                        nc.vector.tensor_add(acc[:], sl, t[:, D : 2 * D])
                        first = False
                    elif i == 0 and n == 1:
                        continue  # consumed by the first add
                    else:
                        nc.vector.tensor_add(acc[:], acc[:], sl)

            # Partition reduce + broadcast: out[p, f] = sum_k acc[k, f] for all p
            ps = psum_pool.tile([P, D], f32)
            nc.tensor.matmul(ps[:, 0:512], ones[:], acc[:, 0:512], start=True, stop=True)
            nc.tensor.matmul(ps[:, 512:D], ones[:], acc[:, 512:D], start=True, stop=True)

            # Replicate along free dim (log doubling) to get 4 MB write tiles
            bc = bcast_pool.tile([P, OUT_REP * D], f32)
            nc.vector.tensor_copy(bc[:, 0:D], ps[:])
            rep = 1
            while rep < OUT_REP:
                nc.vector.tensor_copy(bc[:, rep * D : 2 * rep * D], bc[:, 0 : rep * D])
                rep *= 2

            bc_re = bc[:].rearrange("p (n m) -> p n m", n=OUT_REP)
            for i in range(N_OUT):
                nc.sync.dma_start(o_re[i], bc_re)

    nc.compile()
    return nc


def _get_program():
    if "nc" not in _CACHE:
        _CACHE["nc"] = _build_program()
    return _CACHE["nc"]


def kernel(query: np.ndarray, value: np.ndarray) -> np.ndarray:
    from concourse.bass_utils import run_bass_kernel_spmd

    del query  # output is exactly independent of query (see module docstring)
    value = np.ascontiguousarray(value, dtype=np.float32)
    assert value.shape == (B, S, D)

    nc = _get_program()
    in_maps = [{"value": value[b]} for b in range(B)]
    res = run_bass_kernel_spmd(nc, in_maps, list(range(B)))
    return np.stack([res.results[b]["out"] for b in range(B)], axis=0)
